# revision 4
# baseline (speedup 1.0000x reference)
"""MoE top-2 SwiGLU FFN kernel for 8 Trainium2 NeuronCores.

Strategy (expert-parallel, host-side dispatch):
  - The gate (x @ Wg, top-2, softmax) is tiny; it is computed on the host
    in float64 and used to dispatch tokens to experts ("all-to-all by
    top-k indices" done during input sharding).
  - One expert per core. Each core runs a dense SwiGLU FFN over the
    tokens routed to its expert (padded to a common capacity C):
        Ht1 = W1c.T @ Xt   (per 128-chunk of H, accumulated over D)
        Ht3 = W3c.T @ Xt
        Gt  = silu(Ht1) * Ht3                  [H, C] kept in SBUF (bf16)
        Yt  = sum_hk W2[hk].T-chunks @ Gt[hk]  [D, C]
    Everything is computed in the "transposed" orientation (tokens always
    in the matmul moving/free dimension) so no on-chip transposes are
    needed and C needs no 128-alignment.
  - Host applies the softmax combine weights and scatter-adds the two
    expert contributions per token.

  Matmuls run in bf16 (fp32 PSUM accumulation). Weights are pre-packed
  on the host into layouts that give fully-contiguous per-partition DMA.
"""

import numpy as np
import ml_dtypes

E = 8          # experts == cores
D = 2048       # model dim
H = 7168       # hidden dim
P = 128
DK = D // P    # 16 contraction chunks for stage 1
HK = H // P    # 56 hidden chunks
DT = D // P    # 16 output d-tiles for stage 2
TOP_K = 2

BF16 = ml_dtypes.bfloat16

_built_cache = {}


def _chunks(C):
    """Split token dim C into matmul moving-dim chunks (<=512 each)."""
    out = []
    c0 = 0
    while c0 < C:
        cn = min(512, C - c0)
        out.append((c0, cn))
        c0 += cn
    return out


def _build(C):
    """Build + compile the per-core SPMD Bass program for capacity C."""
    if C in _built_cache:
        return _built_cache[C]

    import concourse.tile as tile
    import concourse.mybir as mybir
    from concourse import bacc
    from contextlib import ExitStack

    bf16 = mybir.dt.bfloat16
    f32 = mybir.dt.float32

    nc = bacc.Bacc("TRN2", target_bir_lowering=False, debug=False)

    xt = nc.dram_tensor("xt", [DK, P, C], bf16, kind="ExternalInput").ap()
    w1 = nc.dram_tensor("w1", [HK, P, DK, P], bf16, kind="ExternalInput").ap()
    w3 = nc.dram_tensor("w3", [HK, P, DK, P], bf16, kind="ExternalInput").ap()
    w2 = nc.dram_tensor("w2", [DT, P, HK, P], bf16, kind="ExternalInput").ap()
    yt = nc.dram_tensor("yt", [DT, P, C], f32, kind="ExternalOutput").ap()

    chunks = _chunks(C)

    with tile.TileContext(nc) as tc, ExitStack() as ctx:
        xpool = ctx.enter_context(tc.tile_pool(name="xpool", bufs=1))
        gpool = ctx.enter_context(tc.tile_pool(name="gpool", bufs=1))
        wpool = ctx.enter_context(tc.tile_pool(name="wpool", bufs=3))
        spool = ctx.enter_context(tc.tile_pool(name="spool", bufs=3))

        # Stage 0: load all of Xt (16 tiles of [128, C]) once. The first
        # hk iteration's weights are DMA'd per-dk (interleaved with the
        # xt tiles) so the first matmul starts after ~0.5MB, not ~3.5MB.
        xts = []
        for dk in range(DK):
            xtile = xpool.tile([P, C], bf16, name=f"xt{dk}", tag=f"xt{dk}")
            xts.append(xtile)

        # Stage 1: Gt[hk] = silu(W1[:,hk].T @ X.T) * (W3[:,hk].T @ X.T)
        gts = []
        with tc.tile_pool(name="ps1", bufs=2, space="PSUM") as ps1:
            for hk in range(HK):
                w1t = wpool.tile([P, DK, P], bf16, name=f"w1t{hk}", tag="w1t")
                w3t = wpool.tile([P, DK, P], bf16, name=f"w3t{hk}", tag="w3t")
                if hk == 0:
                    for dk in range(DK):
                        nc.sync.dma_start(out=w1t[:, dk, :], in_=w1[0, :, dk, :])
                        nc.sync.dma_start(out=w3t[:, dk, :], in_=w3[0, :, dk, :])
                        nc.sync.dma_start(out=xts[dk][:], in_=xt[dk])
                else:
                    nc.sync.dma_start(out=w1t[:], in_=w1[hk])
                    nc.sync.dma_start(out=w3t[:], in_=w3[hk])

                h1 = ps1.tile([P, C], f32, name=f"h1_{hk}", tag="h1")
                h3 = ps1.tile([P, C], f32, name=f"h3_{hk}", tag="h3")
                for dk in range(DK):
                    st = dk == 0
                    sp = dk == DK - 1
                    for (c0, cn) in chunks:
                        nc.tensor.matmul(
                            h1[:, c0:c0 + cn], w1t[:, dk, :],
                            xts[dk][:, c0:c0 + cn], start=st, stop=sp,
                        )
                    for (c0, cn) in chunks:
                        nc.tensor.matmul(
                            h3[:, c0:c0 + cn], w3t[:, dk, :],
                            xts[dk][:, c0:c0 + cn], start=st, stop=sp,
                        )
                s1 = spool.tile([P, C], bf16, name=f"s1_{hk}", tag="s1")
                nc.scalar.activation(
                    s1[:], h1[:], mybir.ActivationFunctionType.Silu
                )
                g = gpool.tile([P, C], bf16, name=f"gt{hk}", tag=f"gt{hk}")
                nc.vector.tensor_mul(g[:], s1[:], h3[:])
                gts.append(g)

        # Stage 2: Yt[dt] = sum_hk W2[hk, dt-slice].T @ Gt[hk]
        w2pool = ctx.enter_context(tc.tile_pool(name="w2pool", bufs=2))
        ypool = ctx.enter_context(tc.tile_pool(name="ypool", bufs=3))
        with tc.tile_pool(name="ps2", bufs=2, space="PSUM") as ps2:
            for dt in range(DT):
                w2t = w2pool.tile([P, HK, P], bf16, name=f"w2t{dt}", tag="w2t")
                nc.sync.dma_start(out=w2t[:], in_=w2[dt])
                yp = ps2.tile([P, C], f32, name=f"yp{dt}", tag="yp")
                for hk in range(HK):
                    st = hk == 0
                    sp = hk == HK - 1
                    for (c0, cn) in chunks:
                        nc.tensor.matmul(
                            yp[:, c0:c0 + cn], w2t[:, hk, :],
                            gts[hk][:, c0:c0 + cn], start=st, stop=sp,
                        )
                yo = ypool.tile([P, C], f32, name=f"yo{dt}", tag="yo")
                nc.scalar.copy(yo[:], yp[:])
                nc.sync.dma_start(out=yt[dt], in_=yo[:])

    nc.compile()
    _built_cache[C] = nc
    return nc


def kernel(x, Wg, w1, w3, w2):
    from concourse.bass_utils import run_bass_kernel_spmd

    Bs, Ss, Dd = x.shape
    T = Bs * Ss
    xf = np.ascontiguousarray(x.reshape(T, Dd).astype(np.float32))

    # ---- host gate: scores, top-2, softmax (float64 for stability) ----
    scores = xf.astype(np.float64) @ np.asarray(Wg, np.float64)
    ei = np.argpartition(-scores, TOP_K - 1, axis=1)[:, :TOP_K]
    row = np.arange(T)[:, None]
    sv = scores[row, ei]
    order = np.argsort(-sv, axis=1)
    ei = ei[row, order]                     # [T, 2] expert ids, desc score
    sv = sv[row, order]
    svm = sv - sv.max(axis=1, keepdims=True)
    esv = np.exp(svm)
    cw = (esv / esv.sum(axis=1, keepdims=True)).astype(np.float32)  # [T, 2]

    # ---- dispatch: token lists per expert ----
    idx = [np.where((ei == e).any(axis=1))[0] for e in range(E)]
    wts = []
    for e in range(E):
        sel = ei[idx[e]]
        k = (sel == e).argmax(axis=1)
        wts.append(cw[idx[e], k])
    counts = [len(i) for i in idx]
    C = max(64, -(-max(counts) // 8) * 8)  # round up to multiple of 8

    nc = _build(C)

    # ---- pack per-core inputs ----
    in_maps = []
    w1a, w3a, w2a = np.asarray(w1), np.asarray(w3), np.asarray(w2)
    for e in range(E):
        xe = np.zeros((C, D), np.float32)
        xe[: counts[e]] = xf[idx[e]]
        xtp = np.ascontiguousarray(xe.T.astype(BF16)).reshape(DK, P, C)
        w1p = np.ascontiguousarray(
            w1a[e].astype(BF16).reshape(DK, P, HK, P).transpose(2, 1, 0, 3)
        )
        w3p = np.ascontiguousarray(
            w3a[e].astype(BF16).reshape(DK, P, HK, P).transpose(2, 1, 0, 3)
        )
        w2p = np.ascontiguousarray(
            w2a[e].astype(BF16).reshape(HK, P, DT, P).transpose(2, 1, 0, 3)
        )
        in_maps.append({"xt": xtp, "w1": w1p, "w3": w3p, "w2": w2p})

    import os
    trace = bool(os.environ.get("MOE_TRACE"))
    res = run_bass_kernel_spmd(nc, in_maps, list(range(E)), trace=trace)
    global _last_results
    _last_results = res

    # ---- combine: y[t] += cw[t, e] * FFN_e(x[t]) ----
    y = np.zeros((T, D), np.float32)
    for e in range(E):
        yte = res.results[e]["yt"].reshape(D, C)
        ye = yte.T[: counts[e]]
        y[idx[e]] += wts[e][:, None] * ye
    return y.reshape(Bs, Ss, Dd).astype(x.dtype)


# revision 5
# speedup vs baseline: 1.1937x; 1.1937x over previous
"""MoE top-2 SwiGLU FFN kernel for 8 Trainium2 NeuronCores.

Strategy (expert-parallel, host-side dispatch):
  - The gate (x @ Wg, top-2, softmax) is tiny; it is computed on the host
    in float64 and used to dispatch tokens to experts ("all-to-all by
    top-k indices" done during input sharding).
  - One expert per core. Each core runs a dense SwiGLU FFN over the
    tokens routed to its expert (padded to a common capacity C):
        Ht1 = W1c.T @ Xt   (per 128-chunk of H, accumulated over D)
        Ht3 = W3c.T @ Xt
        Gt  = silu(Ht1) * Ht3                  [H, C] kept in SBUF (bf16)
        Yt  = sum_hk W2[hk].T-chunks @ Gt[hk]  [D, C]
    Everything is computed in the "transposed" orientation (tokens always
    in the matmul moving/free dimension) so no on-chip transposes are
    needed and C needs no 128-alignment.
  - Host applies the softmax combine weights and scatter-adds the two
    expert contributions per token.

  Matmuls run in bf16 (fp32 PSUM accumulation). Weights are pre-packed
  on the host into layouts that give fully-contiguous per-partition DMA.
"""

import numpy as np
import ml_dtypes

E = 8          # experts == cores
D = 2048       # model dim
H = 7168       # hidden dim
P = 128
DK = D // P    # 16 contraction chunks for stage 1
HK = H // P    # 56 hidden chunks
DT = D // P    # 16 output d-tiles for stage 2
TOP_K = 2

BF16 = ml_dtypes.bfloat16

_built_cache = {}


def _chunks(C):
    """Split token dim C into matmul moving-dim chunks (<=512 each)."""
    out = []
    c0 = 0
    while c0 < C:
        cn = min(512, C - c0)
        out.append((c0, cn))
        c0 += cn
    return out


def _build(C):
    """Build + compile the per-core SPMD Bass program for capacity C.

    SBUF/PSUM tiles are allocated with a 64-token-aligned pitch CP (keeps
    partition rows 128B-aligned for full-rate PE streaming); only the
    first C columns are computed."""
    if C in _built_cache:
        return _built_cache[C]
    CP = -(-C // 64) * 64

    import concourse.tile as tile
    import concourse.mybir as mybir
    from concourse import bacc
    from contextlib import ExitStack

    bf16 = mybir.dt.bfloat16
    f32 = mybir.dt.float32

    nc = bacc.Bacc("TRN2", target_bir_lowering=False, debug=False)

    xt = nc.dram_tensor("xt", [DK, P, C], bf16, kind="ExternalInput").ap()
    w1 = nc.dram_tensor("w1", [HK, P, DK, P], bf16, kind="ExternalInput").ap()
    w3 = nc.dram_tensor("w3", [HK, P, DK, P], bf16, kind="ExternalInput").ap()
    w2 = nc.dram_tensor("w2", [DT, P, HK, P], bf16, kind="ExternalInput").ap()
    yt = nc.dram_tensor("yt", [DT, P, C], f32, kind="ExternalOutput").ap()

    chunks = _chunks(C)

    with tile.TileContext(nc) as tc, ExitStack() as ctx:
        xpool = ctx.enter_context(tc.tile_pool(name="xpool", bufs=1))
        gpool = ctx.enter_context(tc.tile_pool(name="gpool", bufs=1))
        wpool = ctx.enter_context(tc.tile_pool(name="wpool", bufs=3))
        spool = ctx.enter_context(tc.tile_pool(name="spool", bufs=3))

        # Stage 0: load all of Xt (16 tiles of [128, C]) once. The first
        # hk iteration's weights are DMA'd per-dk (interleaved with the
        # xt tiles) so the first matmul starts after ~0.5MB, not ~3.5MB.
        xts = []
        for dk in range(DK):
            xtile = xpool.tile([P, CP], bf16, name=f"xt{dk}", tag=f"xt{dk}")
            xts.append(xtile)

        # Stage 1: Gt[hk] = silu(W1[:,hk].T @ X.T) * (W3[:,hk].T @ X.T)
        gts = []
        with tc.tile_pool(name="ps1", bufs=2, space="PSUM") as ps1:
            for hk in range(HK):
                w1t = wpool.tile([P, DK, P], bf16, name=f"w1t{hk}", tag="w1t")
                w3t = wpool.tile([P, DK, P], bf16, name=f"w3t{hk}", tag="w3t")
                if hk == 0:
                    for dk in range(DK):
                        nc.sync.dma_start(out=w1t[:, dk, :], in_=w1[0, :, dk, :])
                        nc.sync.dma_start(out=w3t[:, dk, :], in_=w3[0, :, dk, :])
                        nc.sync.dma_start(out=xts[dk][:, :C], in_=xt[dk])
                else:
                    nc.sync.dma_start(out=w1t[:], in_=w1[hk])
                    nc.sync.dma_start(out=w3t[:], in_=w3[hk])

                h1 = ps1.tile([P, CP], f32, name=f"h1_{hk}", tag="h1")
                h3 = ps1.tile([P, CP], f32, name=f"h3_{hk}", tag="h3")
                for dk in range(DK):
                    st = dk == 0
                    sp = dk == DK - 1
                    for (c0, cn) in chunks:
                        nc.tensor.matmul(
                            h1[:, c0:c0 + cn], w1t[:, dk, :],
                            xts[dk][:, c0:c0 + cn], start=st, stop=sp,
                        )
                    for (c0, cn) in chunks:
                        nc.tensor.matmul(
                            h3[:, c0:c0 + cn], w3t[:, dk, :],
                            xts[dk][:, c0:c0 + cn], start=st, stop=sp,
                        )
                s1 = spool.tile([P, CP], bf16, name=f"s1_{hk}", tag="s1")
                nc.scalar.activation(
                    s1[:, :C], h1[:, :C], mybir.ActivationFunctionType.Silu
                )
                g = gpool.tile([P, CP], bf16, name=f"gt{hk}", tag=f"gt{hk}")
                nc.vector.tensor_mul(g[:, :C], s1[:, :C], h3[:, :C])
                gts.append(g)

        # Stage 2: Yt[dt] = sum_hk W2[hk, dt-slice].T @ Gt[hk]
        w2pool = ctx.enter_context(tc.tile_pool(name="w2pool", bufs=2))
        ypool = ctx.enter_context(tc.tile_pool(name="ypool", bufs=3))
        with tc.tile_pool(name="ps2", bufs=2, space="PSUM") as ps2:
            for dt in range(DT):
                w2t = w2pool.tile([P, HK, P], bf16, name=f"w2t{dt}", tag="w2t")
                nc.sync.dma_start(out=w2t[:], in_=w2[dt])
                yp = ps2.tile([P, CP], f32, name=f"yp{dt}", tag="yp")
                for hk in range(HK):
                    st = hk == 0
                    sp = hk == HK - 1
                    for (c0, cn) in chunks:
                        nc.tensor.matmul(
                            yp[:, c0:c0 + cn], w2t[:, hk, :],
                            gts[hk][:, c0:c0 + cn], start=st, stop=sp,
                        )
                yo = ypool.tile([P, CP], f32, name=f"yo{dt}", tag="yo")
                nc.scalar.copy(yo[:, :C], yp[:, :C])
                nc.sync.dma_start(out=yt[dt], in_=yo[:, :C])

    nc.compile()
    _built_cache[C] = nc
    return nc


def kernel(x, Wg, w1, w3, w2):
    from concourse.bass_utils import run_bass_kernel_spmd

    Bs, Ss, Dd = x.shape
    T = Bs * Ss
    xf = np.ascontiguousarray(x.reshape(T, Dd).astype(np.float32))

    # ---- host gate: scores, top-2, softmax (float64 for stability) ----
    scores = xf.astype(np.float64) @ np.asarray(Wg, np.float64)
    ei = np.argpartition(-scores, TOP_K - 1, axis=1)[:, :TOP_K]
    row = np.arange(T)[:, None]
    sv = scores[row, ei]
    order = np.argsort(-sv, axis=1)
    ei = ei[row, order]                     # [T, 2] expert ids, desc score
    sv = sv[row, order]
    svm = sv - sv.max(axis=1, keepdims=True)
    esv = np.exp(svm)
    cw = (esv / esv.sum(axis=1, keepdims=True)).astype(np.float32)  # [T, 2]

    # ---- dispatch: token lists per expert ----
    idx = [np.where((ei == e).any(axis=1))[0] for e in range(E)]
    wts = []
    for e in range(E):
        sel = ei[idx[e]]
        k = (sel == e).argmax(axis=1)
        wts.append(cw[idx[e], k])
    counts = [len(i) for i in idx]
    C = max(64, -(-max(counts) // 8) * 8)  # round up to multiple of 8

    nc = _build(C)

    # ---- pack per-core inputs ----
    in_maps = []
    w1a, w3a, w2a = np.asarray(w1), np.asarray(w3), np.asarray(w2)
    for e in range(E):
        xe = np.zeros((C, D), np.float32)
        xe[: counts[e]] = xf[idx[e]]
        xtp = np.ascontiguousarray(xe.T.astype(BF16)).reshape(DK, P, C)
        w1p = np.ascontiguousarray(
            w1a[e].astype(BF16).reshape(DK, P, HK, P).transpose(2, 1, 0, 3)
        )
        w3p = np.ascontiguousarray(
            w3a[e].astype(BF16).reshape(DK, P, HK, P).transpose(2, 1, 0, 3)
        )
        w2p = np.ascontiguousarray(
            w2a[e].astype(BF16).reshape(HK, P, DT, P).transpose(2, 1, 0, 3)
        )
        in_maps.append({"xt": xtp, "w1": w1p, "w3": w3p, "w2": w2p})

    import os
    trace = bool(os.environ.get("MOE_TRACE"))
    res = run_bass_kernel_spmd(nc, in_maps, list(range(E)), trace=trace)
    global _last_results
    _last_results = res

    # ---- combine: y[t] += cw[t, e] * FFN_e(x[t]) ----
    y = np.zeros((T, D), np.float32)
    for e in range(E):
        yte = res.results[e]["yt"].reshape(D, C)
        ye = yte.T[: counts[e]]
        y[idx[e]] += wts[e][:, None] * ye
    return y.reshape(Bs, Ss, Dd).astype(x.dtype)
